# revision 9
# baseline (speedup 1.0000x reference)
"""PairRepresentation kernel for 8x Trainium2 NeuronCores (Bass/Tile).

Math: out[b,i,j,:] = (left[i] + right[j] + E[j-i+2048]) @ Wo + bo
with left = x@Wl + bl, right = x@Wr + br.

Since the projection distributes over the sum:
  out[i,j] = x[i]@(Wl@Wo) + x[j]@(Wr@Wo) + E[j-i+2048]@Wo
             + ((bl+br)@Wo + bo)

Host-side prep folds the weight-weight products (Wl@Wo, Wr@Wo, E_win@Wo,
bias sums) -- pure weight preprocessing; every x-dependent FLOP runs on
device. Each core owns 96 consecutive i-rows (sequence parallel over the
first L axis). On device, everything is channel-on-partitions:
  RpT [256, 768]  = (Wr@Wo)^T @ xT            (all j, replicated work)
  LpT [256, 96]   = (Wl@Wo)^T @ xT_own + bias (own i rows)
  EpT [256, 864]  = input: per-core 863-wide rel-pos window @ Wo, transposed
Main loop, per (8-row batch, channel half):
  one DVE tensor_tensor  S[c, bi, j] = EpT[c, 95-i+j] + RpT[c, j]
    (diagonal stride -1 view of EpT, broadcast view of RpT)
  eight in-place ScalarE bias adds   S[c, bi, :] += LpT[c, i]
  two 1.5MB DMA writes to transposed DRAM outT[c, i, j], issued as soon
  as their 4 rows are biased (single HWDGE ring; two rings fragment the
  HBM write stream).
The host gather transposes back to (1, L, L, 256).
"""

import sys

for p in ("/opt/trn_rl_repo", "/root/.axon_site/_ro/trn_rl_repo"):
    if p not in sys.path:
        sys.path.append(p)

import numpy as np

import concourse.bass as bass
import concourse.tile as tile
from concourse import bacc, mybir
from concourse.bass import ts
from concourse.bass_utils import run_bass_kernel_spmd

N_CORES = 8
L = 768
D = 256
DP = 64
MAX_LEN = 2048
LPC = L // N_CORES  # 96 i-rows per core
IB = 8              # i-rows per batch
NB = LPC // IB      # 12 batches
EW = LPC - 1 + L    # 863: per-core rel-pos window width
EWP = 864           # padded to even
F32 = mybir.dt.float32

_CACHED_NC = None
_last_in_maps = None


def _build_nc():
    nc = bacc.Bacc("TRN2", target_bir_lowering=False, debug=False,
                   num_devices=N_CORES)

    xT_d = nc.dram_tensor("xT", [D, L], F32, kind="ExternalInput")
    xo_d = nc.dram_tensor("xT_own", [D, LPC], F32, kind="ExternalInput")
    Wro_d = nc.dram_tensor("Wro", [D, D], F32, kind="ExternalInput")
    Wlo_d = nc.dram_tensor("Wlo", [D, D], F32, kind="ExternalInput")
    ep_d = nc.dram_tensor("EpT", [D, EWP], F32, kind="ExternalInput")
    bs_d = nc.dram_tensor("bsum", [D, 1], F32, kind="ExternalInput")
    out_d = nc.dram_tensor("outT", [D, LPC, L], F32, kind="ExternalOutput")

    with tile.TileContext(nc) as tc:
        with (
            tc.tile_pool(name="consts", bufs=1) as cp,
            tc.tile_pool(name="psum", bufs=2, space=bass.MemorySpace.PSUM) as pp,
            tc.tile_pool(name="work", bufs=3) as wp,
        ):
            # ---- load inputs (channel halves on partitions) ----
            # order matters: xT/Wro gate RpT which gates the main loop.
            xTt = [cp.tile([128, L], F32, name=f"xT{k}", tag=f"xT{k}") for k in range(2)]
            Wrot = [cp.tile([128, D], F32, name=f"Wro{k}", tag=f"Wro{k}") for k in range(2)]
            EpT = [cp.tile([128, EWP], F32, name=f"Ep{h}", tag=f"Ep{h}") for h in range(2)]
            xot = [cp.tile([128, LPC], F32, name=f"xo{k}", tag=f"xo{k}") for k in range(2)]
            Wlot = [cp.tile([128, D], F32, name=f"Wlo{k}", tag=f"Wlo{k}") for k in range(2)]
            bst = [cp.tile([128, 1], F32, name=f"bs{h}", tag=f"bs{h}") for h in range(2)]
            for k in range(2):
                nc.sync.dma_start(out=xTt[k][:], in_=xT_d[ts(k, 128), :])
                nc.sync.dma_start(out=Wrot[k][:], in_=Wro_d[ts(k, 128), :])
            for h in range(2):
                nc.sync.dma_start(out=EpT[h][:], in_=ep_d[ts(h, 128), :])
            for k in range(2):
                nc.sync.dma_start(out=xot[k][:], in_=xo_d[ts(k, 128), :])
                nc.sync.dma_start(out=Wlot[k][:], in_=Wlo_d[ts(k, 128), :])
                nc.sync.dma_start(out=bst[k][:], in_=bs_d[ts(k, 128), :])

            # ---- tiny projections on PE (fp32, K=256 accumulated) ----
            RpT = [cp.tile([128, L], F32, name=f"Rp{h}", tag=f"Rp{h}") for h in range(2)]
            LpT = [cp.tile([128, LPC], F32, name=f"Lp{h}", tag=f"Lp{h}") for h in range(2)]
            for h in range(2):
                for n in range(0, L, 384):
                    ps = pp.tile([128, 384], F32, name="ps", tag="ps")
                    nc.tensor.matmul(ps[:], Wrot[0][:, ts(h, 128)],
                                     xTt[0][:, n:n + 384], start=True, stop=False)
                    nc.tensor.matmul(ps[:], Wrot[1][:, ts(h, 128)],
                                     xTt[1][:, n:n + 384], start=False, stop=True)
                    nc.scalar.copy(RpT[h][:, n:n + 384], ps[:])
            for h in range(2):
                ps = pp.tile([128, LPC], F32, name="ps", tag="ps")
                nc.tensor.matmul(ps[:], Wlot[0][:, ts(h, 128)], xot[0][:],
                                 start=True, stop=False)
                nc.tensor.matmul(ps[:], Wlot[1][:, ts(h, 128)], xot[1][:],
                                 start=False, stop=True)
                # fold (bl+br)@Wo + bo here
                nc.scalar.add(LpT[h][:], ps[:], add=bst[h][:, 0:1])

            # ---- main loop: batched pair sum + output DMA ----
            for b in range(NB):
                for h in range(2):
                    S = wp.tile([128, IB, L], F32, name=f"S{h}", tag=f"S{h}")
                    base = EpT[h][:, LPC - 1 - b * IB:]
                    ep_diag = bass.AP(
                        base.tensor, base.offset,
                        [list(base.ap[0]), [-1, IB], [1, L]])
                    rp_bcast = RpT[h][:, None, :].broadcast_to([128, IB, L])
                    nc.vector.tensor_add(S[:], ep_diag, rp_bcast)
                    qn = IB // 2
                    for q in range(2):
                        for bi in range(q * qn, (q + 1) * qn):
                            il = b * IB + bi
                            nc.scalar.add(S[:, bi, :], S[:, bi, :],
                                          add=LpT[h][:, il:il + 1])
                        nc.sync.dma_start(
                            out=out_d[ts(h, 128), b * IB + q * qn:
                                      b * IB + (q + 1) * qn, :],
                            in_=S[:, q * qn:(q + 1) * qn, :])

    nc.compile()
    return nc


def kernel(x, Wl, bl, Wr, br, E, Wo, bo):
    global _CACHED_NC, _last_in_maps
    x = np.asarray(x, dtype=np.float32)
    Wl = np.asarray(Wl, dtype=np.float32)
    bl = np.asarray(bl, dtype=np.float32)
    Wr = np.asarray(Wr, dtype=np.float32)
    br = np.asarray(br, dtype=np.float32)
    E = np.asarray(E, dtype=np.float32)
    Wo = np.asarray(Wo, dtype=np.float32)
    bo = np.asarray(bo, dtype=np.float32)

    B = x.shape[0]
    assert x.shape == (B, L, D) and B == 1

    xT = np.ascontiguousarray(x[0].T)                   # (256, 768)
    # weight-weight folds (no x-dependent compute)
    Wro = np.ascontiguousarray(Wr @ Wo)                 # (256, 256)
    Wlo = np.ascontiguousarray(Wl @ Wo)                 # (256, 256)
    bsum = np.ascontiguousarray(((bl + br) @ Wo + bo).reshape(D, 1))
    # rel-pos rows used: E[2048-767 : 2048+768] -> project through Wo
    EpFullT = (E[MAX_LEN - (L - 1):MAX_LEN + L] @ Wo).T  # (256, 1535)

    in_maps = []
    for c in range(N_CORES):
        i0 = c * LPC
        # core c reads Ep columns w = j - i + (L-1), i in [i0, i0+96),
        # j in [0, 768)  ->  w in [s0, s0+863), s0 = (L-1) - i0 - (LPC-1)
        s0 = (L - 1) - i0 - (LPC - 1)
        epc = np.zeros((D, EWP), dtype=np.float32)
        epc[:, :EW] = EpFullT[:, s0:s0 + EW]
        in_maps.append({
            "xT": xT,
            "xT_own": np.ascontiguousarray(xT[:, i0:i0 + LPC]),
            "Wro": Wro, "Wlo": Wlo,
            "bsum": bsum,
            "EpT": epc,
        })
    _last_in_maps = in_maps

    if _CACHED_NC is None:
        _CACHED_NC = _build_nc()
    nc = _CACHED_NC

    res = run_bass_kernel_spmd(nc, in_maps, list(range(N_CORES)))
    # per-core outT: (256, 96, 768) = [c, i_local, j]
    full = np.concatenate([res.results[c]["outT"] for c in range(N_CORES)],
                          axis=1)                        # (256, 768, 768)
    return np.ascontiguousarray(full.transpose(1, 2, 0))[None]  # (1,768,768,256)


# revision 12
# speedup vs baseline: 1.1879x; 1.1879x over previous
"""PairRepresentation kernel for 8x Trainium2 NeuronCores (Bass/Tile).

Math: out[b,i,j,:] = (left[i] + right[j] + E[j-i+2048]) @ Wo + bo
with left = x@Wl + bl, right = x@Wr + br.

Since the projection distributes over the sum:
  out[i,j] = x[i]@(Wl@Wo) + x[j]@(Wr@Wo) + E[j-i+2048]@Wo
             + ((bl+br)@Wo + bo)

Host-side prep folds the weight-weight products (Wl@Wo, Wr@Wo, E_win@Wo,
bias sums) -- pure weight preprocessing; every x-dependent FLOP runs on
device. Each core owns 96 consecutive i-rows (sequence parallel over the
first L axis). On device, everything is channel-on-partitions:
  RpT [256, 768]  = (Wr@Wo)^T @ xT            (all j, replicated work)
  LpT [256, 96]   = (Wl@Wo)^T @ xT_own + bias (own i rows)
  EpT [256, 864]  = input: per-core 863-wide rel-pos window @ Wo, transposed
Main loop, per (8-row batch, channel half):
  one DVE tensor_tensor  S[c, bi, j] = EpT[c, 95-i+j] + RpT[c, j]
    (diagonal stride -1 view of EpT, broadcast view of RpT)
  eight in-place ScalarE bias adds   S[c, bi, :] += LpT[c, i]
  two 1.5MB DMA writes to transposed DRAM outT[c, i, j], issued as soon
  as their 4 rows are biased, all on the ScalarE HWDGE ring (mixing both
  rings fragments the HBM write stream; the SP ring intermittently
  contends with runtime traffic).
The host gather transposes back to (1, L, L, 256).
"""

import sys

for p in ("/opt/trn_rl_repo", "/root/.axon_site/_ro/trn_rl_repo"):
    if p not in sys.path:
        sys.path.append(p)

import numpy as np

import concourse.bass as bass
import concourse.tile as tile
from concourse import bacc, mybir
from concourse.bass import ts
from concourse.bass_utils import run_bass_kernel_spmd

N_CORES = 8
L = 768
D = 256
DP = 64
MAX_LEN = 2048
LPC = L // N_CORES  # 96 i-rows per core
IB = 8              # i-rows per batch
NB = LPC // IB      # 12 batches
EW = LPC - 1 + L    # 863: per-core rel-pos window width
EWP = 864           # padded to even
F32 = mybir.dt.float32

_CACHED_NC = None
_last_in_maps = None


def _build_nc():
    nc = bacc.Bacc("TRN2", target_bir_lowering=False, debug=False,
                   num_devices=N_CORES)

    xT_d = nc.dram_tensor("xT", [D, L], F32, kind="ExternalInput")
    xo_d = nc.dram_tensor("xT_own", [D, LPC], F32, kind="ExternalInput")
    Wro_d = nc.dram_tensor("Wro", [D, D], F32, kind="ExternalInput")
    Wlo_d = nc.dram_tensor("Wlo", [D, D], F32, kind="ExternalInput")
    ep_d = nc.dram_tensor("EpT", [D, EWP], F32, kind="ExternalInput")
    bs_d = nc.dram_tensor("bsum", [D, 1], F32, kind="ExternalInput")
    out_d = nc.dram_tensor("outT", [D, LPC, L], F32, kind="ExternalOutput")

    with tile.TileContext(nc) as tc:
        with (
            tc.tile_pool(name="consts", bufs=1) as cp,
            tc.tile_pool(name="psum", bufs=2, space=bass.MemorySpace.PSUM) as pp,
            tc.tile_pool(name="work", bufs=3) as wp,
        ):
            # ---- load inputs (channel halves on partitions) ----
            # order matters: xT/Wro gate RpT which gates the main loop.
            xTt = [cp.tile([128, L], F32, name=f"xT{k}", tag=f"xT{k}") for k in range(2)]
            Wrot = [cp.tile([128, D], F32, name=f"Wro{k}", tag=f"Wro{k}") for k in range(2)]
            EpT = [cp.tile([128, EWP], F32, name=f"Ep{h}", tag=f"Ep{h}") for h in range(2)]
            xot = [cp.tile([128, LPC], F32, name=f"xo{k}", tag=f"xo{k}") for k in range(2)]
            Wlot = [cp.tile([128, D], F32, name=f"Wlo{k}", tag=f"Wlo{k}") for k in range(2)]
            bst = [cp.tile([128, 1], F32, name=f"bs{h}", tag=f"bs{h}") for h in range(2)]
            for k in range(2):
                nc.sync.dma_start(out=xTt[k][:], in_=xT_d[ts(k, 128), :])
                nc.sync.dma_start(out=Wrot[k][:], in_=Wro_d[ts(k, 128), :])
            for h in range(2):
                nc.sync.dma_start(out=EpT[h][:], in_=ep_d[ts(h, 128), :])
            for k in range(2):
                nc.sync.dma_start(out=xot[k][:], in_=xo_d[ts(k, 128), :])
                nc.sync.dma_start(out=Wlot[k][:], in_=Wlo_d[ts(k, 128), :])
                nc.sync.dma_start(out=bst[k][:], in_=bs_d[ts(k, 128), :])

            # ---- tiny projections on PE (fp32, K=256 accumulated) ----
            RpT = [cp.tile([128, L], F32, name=f"Rp{h}", tag=f"Rp{h}") for h in range(2)]
            LpT = [cp.tile([128, LPC], F32, name=f"Lp{h}", tag=f"Lp{h}") for h in range(2)]
            for h in range(2):
                for n in range(0, L, 384):
                    ps = pp.tile([128, 384], F32, name="ps", tag="ps")
                    nc.tensor.matmul(ps[:], Wrot[0][:, ts(h, 128)],
                                     xTt[0][:, n:n + 384], start=True, stop=False)
                    nc.tensor.matmul(ps[:], Wrot[1][:, ts(h, 128)],
                                     xTt[1][:, n:n + 384], start=False, stop=True)
                    nc.scalar.copy(RpT[h][:, n:n + 384], ps[:])
            for h in range(2):
                ps = pp.tile([128, LPC], F32, name="ps", tag="ps")
                nc.tensor.matmul(ps[:], Wlot[0][:, ts(h, 128)], xot[0][:],
                                 start=True, stop=False)
                nc.tensor.matmul(ps[:], Wlot[1][:, ts(h, 128)], xot[1][:],
                                 start=False, stop=True)
                # fold (bl+br)@Wo + bo here
                nc.scalar.add(LpT[h][:], ps[:], add=bst[h][:, 0:1])

            # ---- main loop: batched pair sum + output DMA ----
            qn = IB // 2
            for b in range(NB):
                for h in range(2):
                    S = wp.tile([128, IB, L], F32, name=f"S{h}", tag=f"S{h}")
                    # split the very first tile's adds so the output DMA
                    # stream starts a few us earlier
                    tt_halves = 2 if (b == 0 and h == 0) else 1
                    for v in range(tt_halves):
                        vn = IB // tt_halves
                        base = EpT[h][:, LPC - 1 - b * IB - v * vn:]
                        ep_diag = bass.AP(
                            base.tensor, base.offset,
                            [list(base.ap[0]), [-1, vn], [1, L]])
                        rp_bcast = RpT[h][:, None, :].broadcast_to([128, vn, L])
                        nc.vector.tensor_add(S[:, v * vn:(v + 1) * vn, :],
                                             ep_diag, rp_bcast)
                        for q in range(v * vn // qn, (v + 1) * vn // qn):
                            for bi in range(q * qn, (q + 1) * qn):
                                il = b * IB + bi
                                nc.scalar.add(S[:, bi, :], S[:, bi, :],
                                              add=LpT[h][:, il:il + 1])
                            # output DMAs ride the ScalarE HWDGE ring:
                            # the SP ring intermittently contends with
                            # runtime traffic (measured +20us median)
                            nc.scalar.dma_start(
                                out=out_d[ts(h, 128), b * IB + q * qn:
                                          b * IB + (q + 1) * qn, :],
                                in_=S[:, q * qn:(q + 1) * qn, :])

    nc.compile()
    return nc


def kernel(x, Wl, bl, Wr, br, E, Wo, bo):
    global _CACHED_NC, _last_in_maps
    x = np.asarray(x, dtype=np.float32)
    Wl = np.asarray(Wl, dtype=np.float32)
    bl = np.asarray(bl, dtype=np.float32)
    Wr = np.asarray(Wr, dtype=np.float32)
    br = np.asarray(br, dtype=np.float32)
    E = np.asarray(E, dtype=np.float32)
    Wo = np.asarray(Wo, dtype=np.float32)
    bo = np.asarray(bo, dtype=np.float32)

    B = x.shape[0]
    assert x.shape == (B, L, D) and B == 1

    xT = np.ascontiguousarray(x[0].T)                   # (256, 768)
    # weight-weight folds (no x-dependent compute)
    Wro = np.ascontiguousarray(Wr @ Wo)                 # (256, 256)
    Wlo = np.ascontiguousarray(Wl @ Wo)                 # (256, 256)
    bsum = np.ascontiguousarray(((bl + br) @ Wo + bo).reshape(D, 1))
    # rel-pos rows used: E[2048-767 : 2048+768] -> project through Wo
    EpFullT = (E[MAX_LEN - (L - 1):MAX_LEN + L] @ Wo).T  # (256, 1535)

    in_maps = []
    for c in range(N_CORES):
        i0 = c * LPC
        # core c reads Ep columns w = j - i + (L-1), i in [i0, i0+96),
        # j in [0, 768)  ->  w in [s0, s0+863), s0 = (L-1) - i0 - (LPC-1)
        s0 = (L - 1) - i0 - (LPC - 1)
        epc = np.zeros((D, EWP), dtype=np.float32)
        epc[:, :EW] = EpFullT[:, s0:s0 + EW]
        in_maps.append({
            "xT": xT,
            "xT_own": np.ascontiguousarray(xT[:, i0:i0 + LPC]),
            "Wro": Wro, "Wlo": Wlo,
            "bsum": bsum,
            "EpT": epc,
        })
    _last_in_maps = in_maps

    if _CACHED_NC is None:
        _CACHED_NC = _build_nc()
    nc = _CACHED_NC

    res = run_bass_kernel_spmd(nc, in_maps, list(range(N_CORES)))
    # per-core outT: (256, 96, 768) = [c, i_local, j]
    full = np.concatenate([res.results[c]["outT"] for c in range(N_CORES)],
                          axis=1)                        # (256, 768, 768)
    return np.ascontiguousarray(full.transpose(1, 2, 0))[None]  # (1,768,768,256)
